# revision 1
# baseline (speedup 1.0000x reference)
"""Trainium2 Bass kernel for DGMG AddEdge log-prob (gnn_message_passing).

Math restructure (exact in real arithmetic):
    gate = sigmoid(hv @ Wg + bg)                    per node
    s_g  = segment_sum(gate * hv)                   [B, 128]
    sgs  = segment_sum(gate)                        [B]
    logit = s_g @ (Wp @ We_g) + sgs * (bp @ We_g) + hv[last_idx] @ We_s + be
    out  = logsigmoid((2a - 1) * logit)
This avoids materializing proj = hv @ Wp ([N,256]) entirely; the kernel is
memory-bound on streaming hv once.

Sharding: graphs split into 8 contiguous blocks of 1024 (seg_ids sorted); each
core gets the nodes of its graphs (zero-padded to 65536). src rows
(hv[last_idx]) are gathered host-side since last_idx points anywhere in hv.

Device pipeline per core, per 1024-node load tile (64 tiles):
  - gpsimd: scr = hv .* Wg_bcast        (elementwise over [128,1024])
  - DVE:    glog[128,8] = group-wise X-reduce of scr
  - ACT:    gate = sigmoid(glog + bg)
  - DVE:    selg[n,(g,j)] = gate * (segrel[n,g] == j)   (8-seg window/tile)
  - PE:     per 512-half: psum[32,512] = selg_half^T @ hv_half; the valid
            windowed segment partials are the diagonal blocks
            psum[8g:8g+8, 128g:...]; plus gate-sum matmuls vs ones.
  - ACT:    copy psum -> staging SBUF (8 halves pooled per 4-tile round)
  - PE-issued DMA: diagonal blocks -> DRAM virt[4096,129] (row 8*T+j =
    partial sum over 128-node tile T of its window segment b_T+j)
  phase 2 (per 128-graph chunk): every segment is the sum of <=2 virt rows
  (home-tile row; plus next tile's column 0 if cut by a tile boundary):
  indirect-gather both, add, then fused dot products with the folded weights
  and a numerically stable logsigmoid.

All DMA issuance is batched and spread across engines (each dma_start costs
~500ns on its issuing engine).
"""
import copy
import os
import sys

import numpy as np

for _p in ("/opt/trn_rl_repo",):
    if os.path.isdir(_p) and _p not in sys.path:
        sys.path.insert(0, _p)

import bass_rust
import concourse.bass as bass
import concourse.mybir as mybir
import concourse.tile as tile
from concourse.bass_utils import run_bass_kernel_spmd

F32 = mybir.dt.float32
F16 = mybir.dt.float16
I32 = mybir.dt.int32
AL = mybir.AluOpType
AF = mybir.ActivationFunctionType

NCORES = 8
N, B, D, G = 500_000, 8192, 128, 256
BL = B // NCORES           # graphs per core
TIL = 128                  # nodes per window tile
S = 4                      # segment window width per 128-node tile
SS = 8                     # padded window slots per group (cols 4..7 zero)
TILB = 1024                # nodes per load tile
HGRP = TILB // TIL         # 8 groups per load tile
NLT = 64                   # load tiles per core
NP = TILB * NLT            # padded nodes per core (65536)
NTIL = NP // TIL           # 512 window tiles
QROUND = 8                 # load tiles per staging round
NQ = NLT // QROUND         # staging rounds
VROWS = S * NTIL           # 4096
CHUNK = 128
NCH = BL // CHUNK          # 8 phase-2 chunks
PAD_SEGREL = 99.0


def _vrow(T, j):
    """virt row of window-tile T's j-th segment partial. Laid out so each
    drain DMA writes one contiguous 32KB block with 4KB runs (see drains):
    row = 512q + 64*(4h+gg) + 8j + tt for T = 512q//8... (q=T//64,
    tt=(T%64)//8, h=(T%8)//4, gg=T%4)."""
    q = T // 64
    tt = (T % 64) // 8
    h = (T % 8) // 4
    gg = T % 4
    return (8 * 8 * S) * q + (8 * S) * (4 * h + gg) + 8 * j + tt


ZERO_ROW = int(_vrow(NTIL - 1, 0))  # all-zero row (last tile is pure padding)

LAST_RESULTS = None

_WS_CTR = [0]


def split_sync_waits(nc, maxw=1):
    """This walrus build rejects instructions with more than one semaphore
    wait; hoist excess waits onto injected same-engine NoOps."""
    for fn in nc.m.functions:
        for bb in fn.blocks:
            out, changed = [], False
            for inst in bb.instructions:
                si = inst.sync_info
                if si is not None and si.on_wait and len(si.on_wait) > maxw:
                    SI = type(si)
                    waits = list(si.on_wait)
                    extra, keep = waits[:-maxw], waits[-maxw:]
                    for k in range(0, len(extra), maxw):
                        nop = mybir.InstNoOp(
                            name=f"waitsplit_{_WS_CTR[0]}", ins=[], outs=[])
                        _WS_CTR[0] += 1
                        nop.engine = inst.engine
                        nop.bass_nofuse = True
                        nop.sync_info = SI(
                            on_wait=extra[k:k + maxw], on_update=[])
                        out.append(nop)
                    inst.sync_info = SI(
                        on_wait=keep, on_update=list(si.on_update or []))
                    changed = True
                out.append(inst)
            if changed:
                bb.instructions = out
    return nc


def _dram_view(handle, offset_elems, dims):
    """AP over a DRAM tensor with explicit [step, count] dims (element units
    over the row-major flattened tensor)."""
    ap = copy.copy(handle[:, :] if len(handle.shape) > 1 else handle[:])
    ap.offset = offset_elems
    ap.ap = bass_rust.VecI64Pair(dims)
    return ap


def _build(bg0: float, be0: float, c1: float) -> bass.Bass:
    nc = bass.Bass()
    hv_d = nc.declare_dram_parameter("hv", [NLT // 4, TIL, 4 * TILB], F16, isOutput=False)
    sr_d = nc.declare_dram_parameter("segrel", [NQ, TIL, QROUND * S * 2], F32, isOutput=False)
    idx1_d = nc.declare_dram_parameter("idx1", [CHUNK, NCH], I32, isOutput=False)
    idx2_d = nc.declare_dram_parameter("idx2", [CHUNK, NCH], I32, isOutput=False)
    src_d = nc.declare_dram_parameter("src", [CHUNK, NCH * D], F32, isOutput=False)
    sgn_d = nc.declare_dram_parameter("sgn", [CHUNK, NCH], F32, isOutput=False)
    wg_d = nc.declare_dram_parameter("wg8", [TIL, TILB], F32, isOutput=False)
    w1_d = nc.declare_dram_parameter("w1_b", [TIL, D], F32, isOutput=False)
    wes_d = nc.declare_dram_parameter("wes_b", [TIL, D], F32, isOutput=False)
    iota_d = nc.declare_dram_parameter("iota", [TIL, HGRP * SS], F32, isOutput=False)
    ones_d = nc.declare_dram_parameter("ones", [TIL, 1], F16, isOutput=False)
    out_d = nc.declare_dram_parameter("out", [BL, 1], F32, isOutput=True)
    virt_d = nc.dram_tensor("virt", [VROWS, D], F32)
    virts_d = nc.dram_tensor("virts", [VROWS, 1], F32)

    F32R = mybir.dt.float32r
    with tile.TileContext(nc) as tc:
        with (
            tc.tile_pool(name="consts", bufs=1) as cpool,
            tc.tile_pool(name="hvp", bufs=6) as hvpool,
            tc.tile_pool(name="stagep", bufs=2) as stpool,
            tc.tile_pool(name="small", bufs=6) as spool,
            tc.tile_pool(name="scratch", bufs=2) as scpool,
            tc.tile_pool(name="pmain", bufs=3, space="PSUM") as pmain,
            tc.tile_pool(name="psgs", bufs=2, space="PSUM") as psgs,
        ):
            wg_t = cpool.tile([TIL, TILB], F32)
            nc.gpsimd.dma_start(wg_t[:], wg_d[:])
            w1_t = cpool.tile([TIL, D], F32)
            nc.gpsimd.dma_start(w1_t[:], w1_d[:])
            wes_t = cpool.tile([TIL, D], F32)
            nc.gpsimd.dma_start(wes_t[:], wes_d[:])
            iota_t = cpool.tile([TIL, HGRP * SS], F32)
            nc.gpsimd.dma_start(iota_t[:], iota_d[:])
            ones_t = cpool.tile([TIL, 1], F16)
            nc.gpsimd.dma_start(ones_t[:], ones_d[:])
            sgsbuf = cpool.tile([64, NLT], F32)

            for q in range(NQ):
                stage = stpool.tile([64, QROUND * 512], F32, name="stage")
                stage_writes = []
                seg4 = spool.tile([TIL, QROUND * S * 2], F32, name="seg4")
                nc.gpsimd.dma_start(seg4[:], sr_d[q])
                for tt in range(QROUND):
                    t = QROUND * q + tt
                    if t % 4 == 0:
                        hv_big = hvpool.tile([TIL, 4 * TILB], F16, name="hv_big")
                        nc.sync.dma_start(hv_big[:], hv_d[t // 4])
                    hv_t = hv_big[:, TILB * (t % 4):TILB * (t % 4 + 1)]

                    scr = scpool.tile([TIL, TILB], F32, name="scr")
                    nc.gpsimd.tensor_tensor(
                        out=scr[:], in0=hv_t, in1=wg_t[:], op=AL.mult)
                    glog = spool.tile([TIL, HGRP], F32, name="glog")
                    nc.vector.tensor_reduce(
                        out=glog[:],
                        in_=scr[:].rearrange("p (g f) -> p g f", g=HGRP),
                        axis=mybir.AxisListType.X, op=AL.add)
                    gate = spool.tile([TIL, HGRP], F32, name="gate")
                    nc.scalar.activation(gate[:], glog[:], AF.Sigmoid, bias=bg0)

                    sel = spool.tile([TIL, HGRP * SS], F16, name="sel")
                    segt = seg4[:].rearrange(
                        "p (tt2 g2) -> p tt2 g2", tt2=QROUND)[:, tt, :]
                    nc.vector.tensor_tensor(
                        out=sel[:].rearrange("p (g j) -> p g j", g=HGRP),
                        in0=segt.to_broadcast([TIL, HGRP, SS]),
                        in1=iota_t[:].rearrange("p (g j) -> p g j", g=HGRP),
                        op=AL.is_equal,
                    )
                    selg = spool.tile([TIL, HGRP * SS], F16, name="selg")
                    nc.vector.tensor_tensor(
                        out=selg[:].rearrange("p (g j) -> p g j", g=HGRP),
                        in0=sel[:].rearrange("p (g j) -> p g j", g=HGRP),
                        in1=gate[:].to_broadcast([TIL, HGRP, SS]),
                        op=AL.mult,
                    )

                    sgsP = psgs.tile([64, 1], F32, name="sgsP")
                    pm = pmain.tile([64, 512], F32, name="pm")
                    for h in range(2):
                        nc.tensor.matmul(
                            pm[32 * h:32 * (h + 1), :],
                            lhsT=selg[:, 32 * h:32 * (h + 1)],
                            rhs=hv_t[:, 512 * h:512 * (h + 1)],
                            start=True, stop=True)
                    nc.tensor.matmul(
                        sgsP[:], lhsT=selg[:], rhs=ones_t[:],
                        start=True, stop=True)
                    stage4 = stage[:].rearrange(
                        "p (bb tt2 ff) -> p bb tt2 ff", bb=4, tt2=QROUND)
                    stage_writes.append(nc.scalar.activation(
                        stage4[:, :, tt, :],
                        pm[:].rearrange("p (bb ff) -> p bb ff", bb=4),
                        AF.Copy))
                    nc.vector.tensor_copy(sgsbuf[:, t:t + 1], sgsP[:])

                # Drain: per (gg, h), the valid diagonal block rows
                # stage[32h + 8gg + j, gg-block (tt, f)]
                # -> virt[512q + 64*(4h+gg) + 8j + tt, f]: one contiguous
                # 32KB dst block, 8 x 4KB descriptor runs.
                dst6 = virt_d[:].rearrange(
                    "(qq blk j tt) f -> qq blk j tt f",
                    qq=NQ, blk=8, j=S)
                src4 = stage[:].rearrange(
                    "p (bb tt2 ff) -> p bb tt2 ff", bb=4, tt2=QROUND)
                for gg in range(4):
                    for h in range(2):
                        r0 = 32 * h + 8 * gg
                        drain_eng = nc.sync if (gg + h) % 2 == 0 else nc.scalar
                        drain = drain_eng.dma_start(
                            dst6[q, 4 * h + gg], src4[r0:r0 + S, gg, :, :])
                        for wi in stage_writes:
                            tile.add_dep_helper(drain.ins, wi.ins)

            # gate-sum: sgsbuf[32h+8gg+j, 8q+tt] -> virts[vrow(T,j)] for the
            # S valid rows of each (h, gg) block.
            for h in range(2):
                for gg in range(4):
                    sgs_dst = _dram_view(
                        virts_d, (8 * S) * (4 * h + gg),
                        [[8, S], [8 * 8 * S, NQ], [1, QROUND]])
                    nc.gpsimd.dma_start(
                        sgs_dst,
                        sgsbuf[32 * h + 8 * gg:32 * h + 8 * gg + S, :].rearrange(
                            "r (qq tt) -> r qq tt", qq=NQ))

            tc.strict_bb_all_engine_barrier()

            # ---- phase 2 ----
            i1b = spool.tile([CHUNK, NCH], I32, name="i1b")
            nc.gpsimd.dma_start(i1b[:], idx1_d[:])
            i2b = spool.tile([CHUNK, NCH], I32, name="i2b")
            nc.gpsimd.dma_start(i2b[:], idx2_d[:])
            sgnb = spool.tile([CHUNK, NCH], F32, name="sgnb")
            nc.gpsimd.dma_start(sgnb[:], sgn_d[:])
            srcb = stpool.tile([CHUNK, NCH * D], F32, name="srcb")
            nc.sync.dma_start(srcb[:], src_d[:])
            outb = spool.tile([CHUNK, NCH], F32, name="outb")

            for c in range(NCH):
                va = spool.tile([CHUNK, D], F32, name="va")
                nc.gpsimd.indirect_dma_start(
                    out=va[:], out_offset=None, in_=virt_d[:],
                    in_offset=bass.IndirectOffsetOnAxis(ap=i1b[:, c:c + 1], axis=0))
                vb = spool.tile([CHUNK, D], F32, name="vb")
                nc.gpsimd.indirect_dma_start(
                    out=vb[:], out_offset=None, in_=virt_d[:],
                    in_offset=bass.IndirectOffsetOnAxis(ap=i2b[:, c:c + 1], axis=0))
                sg = spool.tile([CHUNK, D], F32, name="sg")
                nc.vector.tensor_add(sg[:], va[:], vb[:])

                scr2 = spool.tile([CHUNK, D], F32, name="scr2")
                nc.vector.tensor_tensor(
                    out=scr2[:], in0=sg[:], in1=w1_t[:], op=AL.mult)
                t1 = spool.tile([CHUNK, 1], F32, name="t1")
                nc.vector.tensor_reduce(
                    out=t1[:], in_=scr2[:], axis=mybir.AxisListType.X, op=AL.add)
                scr3 = spool.tile([CHUNK, D], F32, name="scr3")
                nc.vector.tensor_tensor(
                    out=scr3[:], in0=srcb[:, D * c:D * (c + 1)], in1=wes_t[:],
                    op=AL.mult)
                t2 = spool.tile([CHUNK, 1], F32, name="t2")
                nc.vector.tensor_reduce(
                    out=t2[:], in_=scr3[:], axis=mybir.AxisListType.X, op=AL.add)
                t12 = spool.tile([CHUNK, 1], F32, name="t12")
                nc.vector.tensor_add(t12[:], t1[:], t2[:])
                lg = spool.tile([CHUNK, 1], F32, name="lg")
                nc.vector.tensor_scalar_add(lg[:], t12[:], be0)
                if c1 != 0.0:
                    vas = spool.tile([CHUNK, 1], F32, name="vas")
                    nc.gpsimd.indirect_dma_start(
                        out=vas[:], out_offset=None, in_=virts_d[:],
                        in_offset=bass.IndirectOffsetOnAxis(ap=i1b[:, c:c + 1], axis=0))
                    vbs = spool.tile([CHUNK, 1], F32, name="vbs")
                    nc.gpsimd.indirect_dma_start(
                        out=vbs[:], out_offset=None, in_=virts_d[:],
                        in_offset=bass.IndirectOffsetOnAxis(ap=i2b[:, c:c + 1], axis=0))
                    sgss = spool.tile([CHUNK, 1], F32, name="sgss")
                    nc.vector.tensor_add(sgss[:], vas[:], vbs[:])
                    l3 = spool.tile([CHUNK, 1], F32, name="l3")
                    nc.vector.tensor_scalar_mul(l3[:], sgss[:], c1)
                    lg2 = spool.tile([CHUNK, 1], F32, name="lg2")
                    nc.vector.tensor_add(lg2[:], lg[:], l3[:])
                    lg = lg2

                x = spool.tile([CHUNK, 1], F32, name="x")
                nc.vector.tensor_mul(x[:], lg[:], sgnb[:, c:c + 1])
                mn = spool.tile([CHUNK, 1], F32, name="mn")
                nc.vector.tensor_scalar_min(mn[:], x[:], 0.0)
                mx = spool.tile([CHUNK, 1], F32, name="mx")
                nc.vector.tensor_scalar_max(mx[:], x[:], 0.0)
                nax = spool.tile([CHUNK, 1], F32, name="nax")
                nc.vector.tensor_sub(nax[:], mn[:], mx[:])
                # logsigmoid(x) = min(x,0) - log1p(exp(-|x|))
                e = spool.tile([CHUNK, 1], F32, name="e")
                nc.scalar.activation(e[:], nax[:], AF.Exp)
                lp = spool.tile([CHUNK, 1], F32, name="lp")
                nc.scalar.activation(lp[:], e[:], AF.Ln, bias=1.0)
                nc.vector.tensor_sub(outb[:, c:c + 1], mn[:], lp[:])

            out_dst = out_d[:].rearrange("(c p) one -> p (c one)", p=CHUNK)
            nc.gpsimd.dma_start(out_dst, outb[:])
    return nc


def _prep_core(hv, seg_ids, last_idx, a, m):
    lo = int(np.searchsorted(seg_ids, m * BL, "left"))
    hi = int(np.searchsorted(seg_ids, (m + 1) * BL, "left"))
    nloc = hi - lo
    assert nloc <= NP - TIL, f"core {m}: {nloc} nodes > capacity"
    seg_loc = seg_ids[lo:hi].astype(np.int64) - m * BL
    hv_pad = np.zeros((NP, D), np.float16)
    hv_pad[:nloc] = hv[lo:hi].astype(np.float16)
    hv_p = np.ascontiguousarray(
        hv_pad.reshape(NLT, HGRP, TIL, D).transpose(0, 2, 1, 3)
        .reshape(NLT // 4, 4, TIL, TILB).transpose(0, 2, 1, 3)
        .reshape(NLT // 4, TIL, 4 * TILB))

    nrt = (nloc + TIL - 1) // TIL
    b = np.zeros(NTIL, np.int64)
    b[:nrt] = seg_loc[np.arange(nrt) * TIL]
    segrel = np.full(NP, PAD_SEGREL, np.float32)
    rel = seg_loc - b[np.arange(nloc) // TIL]
    assert rel.min() >= 0 and rel.max() < S, f"window overflow: {rel.max()}"
    segrel[:nloc] = rel
    # [NQ, TIL, QROUND*S]: [q, p, 8*tt + g] = segrel of node 1024*(4q+tt)+128g+p
    sr_p = np.ascontiguousarray(
        segrel.reshape(NQ, QROUND, HGRP, TIL).transpose(0, 3, 1, 2).reshape(
            NQ, TIL, QROUND * HGRP))

    rr = np.arange(BL, dtype=np.int64)
    firsts = np.searchsorted(seg_loc, rr, "left")
    lasts = np.searchsorted(seg_loc, rr + 1, "left")
    nonempty = firsts < lasts
    th = firsts // TIL
    tl = np.maximum(lasts - 1, 0) // TIL
    assert np.all((tl - th)[nonempty] <= 1), "segment spans >2 tiles"
    j1 = rr - b[th]
    assert np.all((j1[nonempty] >= 0) & (j1[nonempty] < S))
    idx1 = np.where(nonempty, _vrow(th, j1), ZERO_ROW).astype(np.int32)
    straddle = nonempty & (tl > th)
    assert np.all(b[tl[straddle]] == rr[straddle])
    idx2 = np.where(straddle, _vrow(tl, 0), ZERO_ROW).astype(np.int32)

    src = hv[last_idx[m * BL:(m + 1) * BL]].astype(np.float32)
    sgn = (2 * a[m * BL:(m + 1) * BL] - 1).astype(np.float32)
    # chunk-major -> [CHUNK, NCH] / [CHUNK, NCH*D]
    idx1_p = np.ascontiguousarray(idx1.reshape(NCH, CHUNK).T)
    idx2_p = np.ascontiguousarray(idx2.reshape(NCH, CHUNK).T)
    sgn_p = np.ascontiguousarray(sgn.reshape(NCH, CHUNK).T)
    src_p = np.ascontiguousarray(
        src.reshape(NCH, CHUNK, D).transpose(1, 0, 2).reshape(CHUNK, NCH * D))
    return hv_p, sr_p, idx1_p, idx2_p, src_p, sgn_p


def prep_all(hv, Wg, bg, Wp, bp, We, be, seg_ids, last_idx, a):
    """Host-side sharding/folding. Returns (in_maps, bg0, be0, c1)."""
    hv = np.asarray(hv, dtype=np.float32)
    Wg = np.asarray(Wg, dtype=np.float32)
    bg = np.asarray(bg, dtype=np.float32)
    Wp = np.asarray(Wp, dtype=np.float32)
    bp = np.asarray(bp, dtype=np.float32)
    We = np.asarray(We, dtype=np.float32)
    be = np.asarray(be, dtype=np.float32)
    seg_ids = np.asarray(seg_ids)
    last_idx = np.asarray(last_idx)
    a = np.asarray(a)

    w1 = (Wp @ We[:G]).astype(np.float32)[:, 0]        # [128]
    wes = We[G:, 0].astype(np.float32)                 # [128]
    c1 = float(bp @ We[:G, 0])
    bg0, be0 = float(bg[0]), float(be[0])

    wg8 = np.ascontiguousarray(
        np.tile(np.broadcast_to(Wg[:, 0][None, :], (TIL, D)), (1, HGRP)), np.float32)
    w1_b = np.ascontiguousarray(np.broadcast_to(w1[None, :], (TIL, D)), np.float32)
    wes_b = np.ascontiguousarray(np.broadcast_to(wes[None, :], (TIL, D)), np.float32)
    slot = np.concatenate([np.arange(S, dtype=np.float32),
                           np.full(SS - S, -1.0, np.float32)])
    iota = np.ascontiguousarray(np.broadcast_to(
        np.tile(slot, HGRP)[None, :], (TIL, HGRP * SS)))
    ones = np.ones((TIL, 1), np.float16)

    in_maps = []
    for m in range(NCORES):
        hv_p, sr_p, idx1, idx2, src, sgn = _prep_core(hv, seg_ids, last_idx, a, m)
        in_maps.append({
            "hv": hv_p, "segrel": sr_p, "idx1": idx1, "idx2": idx2,
            "src": src, "sgn": sgn, "wg8": wg8, "w1_b": w1_b,
            "wes_b": wes_b, "iota": iota, "ones": ones,
        })
    return in_maps, bg0, be0, c1


def kernel(hv, Wg, bg, Wp, bp, We, be, seg_ids, last_idx, a):
    global LAST_RESULTS
    in_maps, bg0, be0, c1 = prep_all(
        hv, Wg, bg, Wp, bp, We, be, seg_ids, last_idx, a)
    nc = _build(bg0, be0, c1)
    split_sync_waits(nc, maxw=1)
    res = run_bass_kernel_spmd(nc, in_maps, core_ids=list(range(NCORES)))
    LAST_RESULTS = res
    out = np.concatenate([np.asarray(res.results[i]["out"]) for i in range(NCORES)], axis=0)
    return out.astype(np.float32)



# revision 5
# speedup vs baseline: 3.1524x; 3.1524x over previous
"""Trainium2 Bass kernel for DGMG AddEdge log-prob (gnn_message_passing).

Math restructure (exact in real arithmetic):
    gate  = sigmoid(hv @ Wg + bg)                       per node
    hdotc = hv @ (Wp @ We_g) + (bp @ We_g)              per node  (feature dot
            folded through the projection; segment_sum commutes with the dot)
    vdot[window-slot] = sum_{n in slot} gate[n] * hdotc[n]
    logit = vdot[home] + vdot[spill] + hv[last_idx] @ We_s + be
    out   = logsigmoid((2a - 1) * logit)
The [B, G] graph embedding is never materialized: only its dot with the
folded head weight survives, so the per-graph quantity is ONE scalar.

Device layout: hv streams in TRANSPOSED f16 tiles [128 features, 1024 nodes].
Per 128-node group g the PE computes out[128 nodes, 2] = hvT_g^T @ [Wg | w1]
(glog and hdotc together, 2-column GEMV), then per-group window GEMVs
vdP[8 slots, 1] = selg_g^T @ gate*hdotc reduce the gated segment sums to
scalars.  ACT does the sigmoid (batched over 4 tiles), DVE builds the
window-selection masks, and the three DMA-capable queues (SP/Act/Pool)
split the hv stream.  Phase 2 gathers two vdot scalars per graph and
applies a numerically stable logsigmoid.

Sharding: graphs split into 8 contiguous blocks of 1024 (seg_ids sorted);
each core gets the nodes of its graphs (zero-padded to 62 x 1024).  src rows
(hv[last_idx]) are gathered host-side since last_idx points anywhere in hv.
"""
import copy
import os
import sys

import numpy as np

for _p in ("/opt/trn_rl_repo",):
    if os.path.isdir(_p) and _p not in sys.path:
        sys.path.insert(0, _p)

import bass_rust
import concourse.bass as bass
import concourse.mybir as mybir
import concourse.tile as tile
from concourse.bass_utils import run_bass_kernel_spmd

F32 = mybir.dt.float32
F16 = mybir.dt.float16
I32 = mybir.dt.int32
AL = mybir.AluOpType
AF = mybir.ActivationFunctionType

NCORES = 8
N, B, D = 500_000, 8192, 128
BL = B // NCORES           # graphs per core
TIL = 128                  # nodes per window tile (= feature count)
TILB = 1024                # nodes per load tile
HGRP = TILB // TIL         # 8 groups per load tile
NLT = 62                   # load tiles per core (max nloc 62761 <= 63488)
NP = TILB * NLT            # padded nodes per core
NTIL = NLT * HGRP          # 496 window tiles (128-node groups)
S = 4                      # valid window slots per 128-node group
SS = 8                     # padded slots (4..7 always zero)
CHUNK = 128
NCH = BL // CHUNK          # 8 phase-2 chunks
QR = 8                     # load tiles per vdP round
VD_ROWS = SS * NTIL        # vdot table rows (3968)
ZIDX = S * NTIL            # row (j=4, T=0): slot 4 is never selected -> 0.0
PAD_SEGREL = 99.0

# const tile column layout: [Wg | w1 | iota(64) | wes(128)]
C_WGW1, C_IOTA, C_WES, C_W = 0, 2, 66, 194

LAST_RESULTS = None

_WS_CTR = [0]


def split_sync_waits(nc, maxw=1):
    """This walrus build rejects instructions with more than one semaphore
    wait; hoist excess waits onto injected same-engine NoOps."""
    for fn in nc.m.functions:
        for bb in fn.blocks:
            out, changed = [], False
            for inst in bb.instructions:
                si = inst.sync_info
                if si is not None and si.on_wait and len(si.on_wait) > maxw:
                    SI = type(si)
                    waits = list(si.on_wait)
                    extra, keep = waits[:-maxw], waits[-maxw:]
                    for k in range(0, len(extra), maxw):
                        nop = mybir.InstNoOp(
                            name=f"waitsplit_{_WS_CTR[0]}", ins=[], outs=[])
                        _WS_CTR[0] += 1
                        nop.engine = inst.engine
                        nop.bass_nofuse = True
                        nop.sync_info = SI(
                            on_wait=extra[k:k + maxw], on_update=[])
                        out.append(nop)
                    inst.sync_info = SI(
                        on_wait=keep, on_update=list(si.on_update or []))
                    changed = True
                out.append(inst)
            if changed:
                bb.instructions = out
    return nc


def _hv_engine_seq(nc):
    """Greedy balance of the 31 hv DMAs over the 3 DMA queues, offset by each
    queue's other phase-1 work (us)."""
    load = {"sync": 1.3, "scalar": 7.0, "gpsimd": 7.4}
    seq = []
    for _ in range(NLT // 2):
        e = min(load, key=load.get)
        load[e] += 1.579
        seq.append(e)
    return [getattr(nc, e) for e in seq]


def _build(bg0: float, be0: float, c1: float) -> bass.Bass:
    nc = bass.Bass()
    hvt_d = nc.declare_dram_parameter(
        "hvt", [NLT // 2, TIL, 2 * TILB], F16, isOutput=False)
    sr_d = nc.declare_dram_parameter("segrel", [TIL, NTIL], F16, isOutput=False)
    cst_d = nc.declare_dram_parameter("cst", [TIL, C_W], F16, isOutput=False)
    src_d = nc.declare_dram_parameter("src", [CHUNK, NCH * D], F16, isOutput=False)
    i12_d = nc.declare_dram_parameter("i12", [CHUNK, 2 * NCH], I32, isOutput=False)
    sgn_d = nc.declare_dram_parameter("sgn", [CHUNK, NCH], F32, isOutput=False)
    out_d = nc.declare_dram_parameter("out", [BL, 1], F32, isOutput=True)
    vdot_d = nc.dram_tensor("vdot", [VD_ROWS, 1], F32)

    eng_seq = _hv_engine_seq(nc)

    with tile.TileContext(nc) as tc:
        with (
            tc.tile_pool(name="consts", bufs=1) as cpool,
            tc.tile_pool(name="hvp", bufs=5) as hvpool,
            tc.tile_pool(name="small", bufs=8) as spool,
            tc.tile_pool(name="pgh", bufs=3, space="PSUM") as ghpool,
            tc.tile_pool(name="pvd", bufs=2, space="PSUM") as vdpool,
        ):
            cst = cpool.tile([TIL, C_W], F16)
            nc.gpsimd.dma_start(cst[:], cst_d[:])
            seg_t = cpool.tile([TIL, NTIL], F16)
            nc.gpsimd.dma_start(seg_t[:], sr_d[:])
            srcb = cpool.tile([CHUNK, NCH * D], F16)
            nc.gpsimd.dma_start(srcb[:], src_d[:])
            i12 = cpool.tile([CHUNK, 2 * NCH], I32)
            nc.gpsimd.dma_start(i12[:], i12_d[:])
            sgnb = cpool.tile([CHUNK, NCH], F32)
            nc.gpsimd.dma_start(sgnb[:], sgn_d[:])
            vdbuf = cpool.tile([SS, NTIL], F32)
            srcd = cpool.tile([CHUNK, NCH], F32)

            wgw1 = cst[:, C_WGW1:C_WGW1 + 2]
            iota3 = cst[:, C_IOTA:C_IOTA + HGRP * SS].rearrange(
                "p (g j) -> p g j", g=HGRP)
            wes_t = cst[:, C_WES:C_WES + D]

            # src-embed dot (independent of hv stream; fills early DVE idle)
            sscr = spool.tile([CHUNK, NCH * D], F16, name="sscr")
            nc.vector.tensor_tensor(
                out=sscr[:].rearrange("p (c f) -> p c f", c=NCH),
                in0=srcb[:].rearrange("p (c f) -> p c f", c=NCH),
                in1=wes_t.rearrange("p (one f) -> p one f", one=1
                                    ).to_broadcast([CHUNK, NCH, D]),
                op=AL.mult)
            nc.vector.tensor_reduce(
                out=srcd[:], in_=sscr[:].rearrange("p (c f) -> p c f", c=NCH),
                axis=mybir.AxisListType.X, op=AL.add)

            # ---- phase 1: stream hv, per-node dots on PE, window GEMVs ----
            rounds = [list(range(r, min(r + 4, NLT))) for r in range(0, NLT, 4)]
            ghP4 = None
            vdP = None
            for rd in rounds:
                r0 = rd[0]
                w = len(rd)
                ghP4 = ghpool.tile([TIL, 16 * w], F32, name="ghP4")
                sels = []
                for tt, t in enumerate(rd):
                    if t % 2 == 0:
                        hv2 = hvpool.tile([TIL, 2 * TILB], F16, name="hv2")
                        eng_seq[t // 2].dma_start(hv2[:], hvt_d[t // 2])
                    hvT = hv2[:, TILB * (t % 2):TILB * (t % 2 + 1)]
                    for g in range(HGRP):
                        nc.tensor.matmul(
                            ghP4[:, 16 * tt + 2 * g:16 * tt + 2 * g + 2],
                            lhsT=hvT[:, TIL * g:TIL * (g + 1)],
                            rhs=wgw1, start=True, stop=True)
                    sel = spool.tile([TIL, HGRP * SS], F16, name="sel")
                    segsl = seg_t[:, HGRP * t:HGRP * (t + 1)].rearrange(
                        "p (g one) -> p g one", g=HGRP)
                    nc.vector.tensor_tensor(
                        out=sel[:].rearrange("p (g j) -> p g j", g=HGRP),
                        in0=segsl.to_broadcast([TIL, HGRP, SS]),
                        in1=iota3, op=AL.is_equal)
                    sels.append(sel)

                ghv = ghP4[:].rearrange("p (x two) -> p x two", two=2)
                gate4 = spool.tile([TIL, 8 * w], F16, name="gate4")
                glog_in = ghv[:, :, 0]
                if bg0 != 0.0:
                    glog_b = spool.tile([TIL, 8 * w], F32, name="glog_b")
                    nc.vector.tensor_scalar_add(glog_b[:], glog_in, bg0)
                    glog_in = glog_b[:]
                nc.scalar.activation(gate4[:], glog_in, AF.Sigmoid)
                hdc4 = spool.tile([TIL, 8 * w], F16, name="hdc4")
                if c1 != 0.0:
                    nc.vector.tensor_scalar_add(hdc4[:], ghv[:, :, 1], c1)
                else:
                    nc.vector.tensor_copy(hdc4[:], ghv[:, :, 1])

                for tt, t in enumerate(rd):
                    if t % QR == 0:
                        vdP = vdpool.tile([SS, 8 * QR], F32, name="vdP")
                    selg = spool.tile([TIL, HGRP * SS], F16, name="selg")
                    nc.vector.tensor_tensor(
                        out=selg[:].rearrange("p (g j) -> p g j", g=HGRP),
                        in0=sels[tt][:].rearrange("p (g j) -> p g j", g=HGRP),
                        in1=gate4[:, 8 * tt:8 * (tt + 1)].rearrange(
                            "p (g one) -> p g one", g=HGRP
                        ).to_broadcast([TIL, HGRP, SS]),
                        op=AL.mult)
                    for g in range(HGRP):
                        col = 8 * (t % QR) + g
                        nc.tensor.matmul(
                            vdP[0:SS, col:col + 1],
                            lhsT=selg[:, SS * g:SS * (g + 1)],
                            rhs=hdc4[:, 8 * tt + g:8 * tt + g + 1],
                            start=True, stop=True)
                    if t % QR == QR - 1 or t == NLT - 1:
                        q = t // QR
                        wq = (t % QR) + 1
                        nc.vector.tensor_copy(
                            vdbuf[:, 8 * QR * q:8 * QR * q + 8 * wq],
                            vdP[0:SS, 0:8 * wq])

            vd_dst = vdot_d[:].rearrange("(j t) one -> j (t one)", j=SS)
            nc.sync.dma_start(vd_dst, vdbuf[:])

            tc.strict_bb_all_engine_barrier()

            # ---- phase 2: gather 2 scalars per graph, logsigmoid ----
            vab = spool.tile([CHUNK, 2 * NCH], F32, name="vab")
            for c in range(2 * NCH):
                nc.gpsimd.indirect_dma_start(
                    out=vab[:, c:c + 1], out_offset=None, in_=vdot_d[:],
                    in_offset=bass.IndirectOffsetOnAxis(
                        ap=i12[:, c:c + 1], axis=0))
            vs = spool.tile([CHUNK, NCH], F32, name="vs")
            nc.vector.tensor_add(vs[:], vab[:, 0:NCH], vab[:, NCH:2 * NCH])
            lg = spool.tile([CHUNK, NCH], F32, name="lg")
            nc.vector.tensor_add(lg[:], vs[:], srcd[:])
            if be0 != 0.0:
                lg2 = spool.tile([CHUNK, NCH], F32, name="lg2")
                nc.vector.tensor_scalar_add(lg2[:], lg[:], be0)
                lg = lg2
            x = spool.tile([CHUNK, NCH], F32, name="x")
            nc.vector.tensor_mul(x[:], lg[:], sgnb[:])
            mn = spool.tile([CHUNK, NCH], F32, name="mn")
            nc.vector.tensor_scalar_min(mn[:], x[:], 0.0)
            mx = spool.tile([CHUNK, NCH], F32, name="mx")
            nc.vector.tensor_scalar_max(mx[:], x[:], 0.0)
            nax = spool.tile([CHUNK, NCH], F32, name="nax")
            nc.vector.tensor_sub(nax[:], mn[:], mx[:])
            # logsigmoid(x) = min(x,0) - log1p(exp(-|x|))
            e = spool.tile([CHUNK, NCH], F32, name="e")
            nc.scalar.activation(e[:], nax[:], AF.Exp)
            lp = spool.tile([CHUNK, NCH], F32, name="lp")
            nc.scalar.activation(lp[:], e[:], AF.Ln, bias=1.0)
            outb = spool.tile([CHUNK, NCH], F32, name="outb")
            nc.vector.tensor_sub(outb[:], mn[:], lp[:])

            out_dst = out_d[:].rearrange("(p c) one -> p (c one)", p=CHUNK)
            nc.sync.dma_start(out_dst, outb[:])
    return nc


def _prep_core(hv16, seg_ids, last_idx, a, m):
    lo = int(np.searchsorted(seg_ids, m * BL, "left"))
    hi = int(np.searchsorted(seg_ids, (m + 1) * BL, "left"))
    nloc = hi - lo
    assert nloc <= NP, f"core {m}: {nloc} nodes > capacity {NP}"
    seg_loc = seg_ids[lo:hi].astype(np.int64) - m * BL

    hv_pad = np.zeros((NP, D), np.float16)
    hv_pad[:nloc] = hv16[lo:hi]
    # [NLT, 1024 nodes, 128 feat] -> transpose -> pair tiles into 2048 cols
    hvt = np.ascontiguousarray(
        hv_pad.reshape(NLT, TILB, D).transpose(0, 2, 1)
        .reshape(NLT // 2, 2, TIL, TILB).transpose(0, 2, 1, 3)
        .reshape(NLT // 2, TIL, 2 * TILB))

    nrt = (nloc + TIL - 1) // TIL
    bT = np.zeros(NTIL, np.int64)
    bT[:nrt] = seg_loc[np.arange(nrt) * TIL]
    segrel = np.full(NP, PAD_SEGREL, np.float32)
    rel = seg_loc - bT[np.arange(nloc) // TIL]
    assert rel.min() >= 0 and rel.max() < S, f"window overflow: {rel.max()}"
    segrel[:nloc] = rel
    sr_p = np.ascontiguousarray(segrel.reshape(NTIL, TIL).T.astype(np.float16))

    rr = np.arange(BL, dtype=np.int64)
    firsts = np.searchsorted(seg_loc, rr, "left")
    lasts = np.searchsorted(seg_loc, rr + 1, "left")
    nonempty = firsts < lasts
    th = firsts // TIL
    tl = np.maximum(lasts - 1, 0) // TIL
    assert np.all((tl - th)[nonempty] <= 1), "segment spans >2 tiles"
    j1 = rr - bT[th]
    assert np.all((j1[nonempty] >= 0) & (j1[nonempty] < S))
    idx1 = np.where(nonempty, NTIL * j1 + th, ZIDX).astype(np.int32)
    straddle = nonempty & (tl > th)
    assert np.all(bT[tl[straddle]] == rr[straddle])
    idx2 = np.where(straddle, tl, ZIDX).astype(np.int32)

    i12 = np.concatenate(
        [idx1.reshape(NCH, CHUNK).T, idx2.reshape(NCH, CHUNK).T],
        axis=1)
    i12 = np.ascontiguousarray(i12, dtype=np.int32)

    src = hv16[last_idx[m * BL:(m + 1) * BL]]
    src_p = np.ascontiguousarray(
        src.reshape(NCH, CHUNK, D).transpose(1, 0, 2).reshape(CHUNK, NCH * D))
    sgn = (2 * a[m * BL:(m + 1) * BL] - 1).astype(np.float32)
    sgn_p = np.ascontiguousarray(sgn.reshape(NCH, CHUNK).T)
    return hvt, sr_p, i12, src_p, sgn_p


def prep_all(hv, Wg, bg, Wp, bp, We, be, seg_ids, last_idx, a):
    """Host-side sharding/folding. Returns (in_maps, bg0, be0, c1)."""
    hv = np.asarray(hv, dtype=np.float32)
    Wg = np.asarray(Wg, dtype=np.float32)
    bg = np.asarray(bg, dtype=np.float32)
    Wp = np.asarray(Wp, dtype=np.float32)
    bp = np.asarray(bp, dtype=np.float32)
    We = np.asarray(We, dtype=np.float32)
    be = np.asarray(be, dtype=np.float32)
    seg_ids = np.asarray(seg_ids)
    last_idx = np.asarray(last_idx)
    a = np.asarray(a)

    G = 2 * D
    w1 = (Wp @ We[:G]).astype(np.float32)[:, 0]        # [128]
    wes = We[G:, 0].astype(np.float32)                 # [128]
    c1 = float(bp @ We[:G, 0])
    bg0, be0 = float(bg[0]), float(be[0])

    cst = np.zeros((TIL, C_W), np.float16)
    cst[:, C_WGW1] = Wg[:, 0].astype(np.float16)
    cst[:, C_WGW1 + 1] = w1.astype(np.float16)
    slot = np.concatenate([np.arange(S, dtype=np.float32),
                           np.full(SS - S, -1.0, np.float32)])
    cst[:, C_IOTA:C_IOTA + HGRP * SS] = np.tile(slot, HGRP)[None, :]
    cst[:, C_WES:C_WES + D] = wes[None, :]

    hv16 = hv.astype(np.float16)
    in_maps = []
    for m in range(NCORES):
        hvt, sr_p, i12, src_p, sgn_p = _prep_core(
            hv16, seg_ids, last_idx, a, m)
        in_maps.append({
            "hvt": hvt, "segrel": sr_p, "cst": cst, "src": src_p,
            "i12": i12, "sgn": sgn_p,
        })
    return in_maps, bg0, be0, c1


def kernel(hv, Wg, bg, Wp, bp, We, be, seg_ids, last_idx, a):
    global LAST_RESULTS
    in_maps, bg0, be0, c1 = prep_all(
        hv, Wg, bg, Wp, bp, We, be, seg_ids, last_idx, a)
    nc = _build(bg0, be0, c1)
    split_sync_waits(nc, maxw=1)
    res = run_bass_kernel_spmd(nc, in_maps, core_ids=list(range(NCORES)))
    LAST_RESULTS = res
    out = np.concatenate(
        [np.asarray(res.results[i]["out"]).reshape(CHUNK, NCH).T.reshape(-1, 1)
         for i in range(NCORES)], axis=0)
    return out.astype(np.float32)


# revision 22
# speedup vs baseline: 4.4978x; 1.4268x over previous
"""Trainium2 Bass kernel for DGMG AddEdge log-prob (gnn_message_passing).

Math restructure (exact in real arithmetic):
    gate  = sigmoid(hv @ Wg + bg)                       per node
    hdotc = hv @ (Wp @ We_g) + (bp @ We_g)              per node  (feature dot
            folded through the projection; segment_sum commutes with the dot)
    vdot[window-slot] = sum_{n in slot} gate[n] * hdotc[n]
    logit = vdot[home] + vdot[spill] + hv[last_idx] @ We_s + be
    out   = logsigmoid((2a - 1) * logit)
The [B, G] graph embedding is never materialized: only its dot with the
folded head weight survives, so the per-graph quantity is ONE scalar.

Device layout: hv streams in TRANSPOSED f16 tiles [128 features, 1024 nodes].
Per 128-node group g the PE computes out[128 nodes, 2] = hvT_g^T @ [Wg | w1]
(glog and hdotc together, 2-column GEMV), then per-group window GEMVs
vdP[4 slots, 1] = selg_g^T @ hdotc reduce the gated segment sums to scalars
that stream to a DRAM vdot table (256B rows for SWDGE dma_gather).  ACT does
the sigmoid (batched over 8 tiles), DVE builds the window-selection masks,
and the three DMA-capable queues (SP/Act/Pool) split the hv stream.  Phase 2
dma_gathers two vdot scalars per graph and applies a stable logsigmoid.

Sharding: graphs split into 8 contiguous blocks of 1024 (seg_ids sorted);
each core gets the nodes of its graphs (zero-padded to 62 x 1024).  src rows
(hv[last_idx]) are gathered host-side since last_idx points anywhere in hv.
"""
import os
import sys

import numpy as np

for _p in ("/opt/trn_rl_repo",):
    if os.path.isdir(_p) and _p not in sys.path:
        sys.path.insert(0, _p)

import concourse.bass as bass
import concourse.mybir as mybir
import concourse.tile as tile
from concourse import library_config
from concourse.bass_utils import run_bass_kernel_spmd
from concourse.library_overlay import lower_extended_insts

F32 = mybir.dt.float32
F16 = mybir.dt.float16
I16 = mybir.dt.int16
AL = mybir.AluOpType
AF = mybir.ActivationFunctionType

NCORES = 8
N, B, D = 500_000, 8192, 128
BL = B // NCORES           # graphs per core
TIL = 128                  # nodes per window tile (= feature count)
TILB = 1024                # nodes per load tile
HGRP = TILB // TIL         # 8 groups per load tile
NLT = 62                   # load tiles per core (max nloc 62761 <= 63488)
NP = TILB * NLT            # padded nodes per core
NTIL = NLT * HGRP          # 496 window tiles (128-node groups)
S = 4                      # window slots per 128-node group
CHUNK = 128
NCH = BL // CHUNK          # 8 phase-2 chunks
GR = 8                     # load tiles per gh/gate round
VR = 16                    # load tiles per vdot-write round
NVR = (NLT + VR - 1) // VR
VD_ROWS = S * NTIL         # vdot table rows (1984)
VW = 64                    # vdot row width (f32) = 256B for dma_gather
PAD_SEGREL = 99.0

# const tile column layout: [Wg | w1 | iota(GR*32) | wes(128)]
C_WGW1, C_IOTA, C_WES, C_W = 0, 2, 2 + GR * 32, 2 + GR * 32 + 128

LAST_RESULTS = None
LAST_NC = None

_WS_CTR = [0]


def split_sync_waits(nc, maxw=1):
    """This walrus build rejects instructions with more than one semaphore
    wait; hoist excess waits onto injected same-engine NoOps."""
    for fn in nc.m.functions:
        for bb in fn.blocks:
            out, changed = [], False
            for inst in bb.instructions:
                si = inst.sync_info
                if si is not None and si.on_wait and len(si.on_wait) > maxw:
                    SI = type(si)
                    waits = list(si.on_wait)
                    extra, keep = waits[:-maxw], waits[-maxw:]
                    for k in range(0, len(extra), maxw):
                        nop = mybir.InstNoOp(
                            name=f"waitsplit_{_WS_CTR[0]}", ins=[], outs=[])
                        _WS_CTR[0] += 1
                        nop.engine = inst.engine
                        nop.bass_nofuse = True
                        nop.sync_info = SI(
                            on_wait=extra[k:k + maxw], on_update=[])
                        out.append(nop)
                    inst.sync_info = SI(
                        on_wait=keep, on_update=list(si.on_update or []))
                    changed = True
                out.append(inst)
            if changed:
                bb.instructions = out
    return nc


def _hv_engine_seq(nc):
    """Greedy balance of the 62 hv DMAs over the 3 DMA queues, offset by each
    queue's other phase-1 work (ns)."""
    load = {"sync": 4300.0, "scalar": 5200.0, "gpsimd": 1900.0}
    per = 790.0
    seq = []
    for _ in range(NLT):
        e = min(load, key=load.get)
        load[e] += per
        seq.append(e)
    return [getattr(nc, e) for e in seq]


def _build(bg0: float, be0: float, c1: float, qn=None) -> bass.Bass:
    """qn: per-gather vdot-write round dependency (16 ints in [0, NVR));
    defaults to fully conservative."""
    if qn is None:
        qn = [NVR - 1] * 16
    nc = bass.Bass()
    hvt_d = nc.declare_dram_parameter("hvt", [NLT, TIL, TILB], F16, isOutput=False)
    sr_d = nc.declare_dram_parameter("segrel", [TIL, NTIL], F16, isOutput=False)
    cst_d = nc.declare_dram_parameter("cst", [TIL, C_W], F16, isOutput=False)
    src_d = nc.declare_dram_parameter("src", [CHUNK, NCH * D], F16, isOutput=False)
    i16_d = nc.declare_dram_parameter("i16t", [CHUNK, 2 * NCH * 8], I16, isOutput=False)
    sgn_d = nc.declare_dram_parameter("sgn", [CHUNK, NCH], F32, isOutput=False)
    out_d = nc.declare_dram_parameter("out", [BL, 1], F32, isOutput=True)
    vdot_d = nc.dram_tensor("vdot", [VD_ROWS, VW], F32)
    vdot3 = vdot_d[:, :].rearrange("(j t) w -> j t w", j=S)

    eng_seq = _hv_engine_seq(nc)

    with tile.TileContext(nc) as tc:
        with (
            tc.tile_pool(name="consts", bufs=1) as cpool,
            tc.tile_pool(name="hvp", bufs=14) as hvpool,
            tc.tile_pool(name="small", bufs=8) as spool,
            tc.tile_pool(name="pgh", bufs=5, space="PSUM") as ghpool,
            tc.tile_pool(name="pvd", bufs=2, space="PSUM") as vdpool,
        ):
            nc.gpsimd.load_library(library_config.mlp)
            cst = cpool.tile([TIL, C_W], F16)
            nc.gpsimd.dma_start(cst[:], cst_d[:])
            seg_t = cpool.tile([TIL, NTIL], F16)
            nc.gpsimd.dma_start(seg_t[:], sr_d[:])
            srcb = cpool.tile([CHUNK, NCH * D], F16)
            nc.sync.dma_start(srcb[:], src_d[:])
            i16t = cpool.tile([CHUNK, 2 * NCH * 8], I16)
            nc.sync.dma_start(i16t[:], i16_d[:])
            sgnb = cpool.tile([CHUNK, NCH], F32)
            nc.sync.dma_start(sgnb[:], sgn_d[:])
            srcd = cpool.tile([CHUNK, NCH], F32)

            wgw1 = cst[:, C_WGW1:C_WGW1 + 2]
            iota3 = cst[:, C_IOTA:C_IOTA + HGRP * S].rearrange(
                "p (g j) -> p g j", g=HGRP)
            wes_t = cst[:, C_WES:C_WES + D]

            # src-embed dot (independent of hv stream; fills early DVE idle)
            sscr = spool.tile([CHUNK, NCH * D], F16, name="sscr")
            nc.vector.tensor_tensor(
                out=sscr[:].rearrange("p (c f) -> p c f", c=NCH),
                in0=srcb[:].rearrange("p (c f) -> p c f", c=NCH),
                in1=wes_t.rearrange("p (one f) -> p one f", one=1
                                    ).to_broadcast([CHUNK, NCH, D]),
                op=AL.mult)
            nc.vector.tensor_reduce(
                out=srcd[:], in_=sscr[:].rearrange("p (c f) -> p c f", c=NCH),
                axis=mybir.AxisListType.X, op=AL.add)

            # ---- phase 1: stream hv, per-node dots on PE, window GEMVs ----
            wdmas = []
            vdP = None
            rounds = [list(range(r, min(r + GR, NLT))) for r in range(0, NLT, GR)]
            for rd in rounds:
                w8 = len(rd)
                ghP = ghpool.tile([TIL, 16 * w8], F32, name="ghP")
                sel8 = spool.tile([TIL, GR * HGRP * S], F16, name="sel8")
                for tt, t in enumerate(rd):
                    hv1 = hvpool.tile([TIL, TILB], F16, name="hv1")
                    eng_seq[t].dma_start(hv1[:], hvt_d[t])
                    for g in range(HGRP):
                        nc.tensor.matmul(
                            ghP[:, 16 * tt + 2 * g:16 * tt + 2 * g + 2],
                            lhsT=hv1[:, TIL * g:TIL * (g + 1)],
                            rhs=wgw1, start=True, stop=True)
                # window-slot selection for the whole round (pre-gate)
                segsl = seg_t[:, HGRP * rd[0]:HGRP * (rd[0] + w8)].rearrange(
                    "p (x one) -> p x one", one=1)
                iot8 = cst[:, C_IOTA:C_IOTA + w8 * HGRP * S].rearrange(
                    "p (x j) -> p x j", j=S)
                nc.vector.tensor_tensor(
                    out=sel8[:, :w8 * HGRP * S].rearrange(
                        "p (x j) -> p x j", j=S),
                    in0=segsl.to_broadcast([TIL, w8 * HGRP, S]),
                    in1=iot8, op=AL.is_equal)

                ghv = ghP[:].rearrange("p (x two) -> p x two", two=2)
                gate8 = spool.tile([TIL, GR * HGRP], F16, name="gate8")
                glog_in = ghv[:, :, 0]
                if bg0 != 0.0:
                    glog_b = spool.tile([TIL, GR * HGRP], F32, name="glog_b")
                    nc.vector.tensor_scalar_add(
                        glog_b[:, :w8 * HGRP], glog_in, bg0)
                    glog_in = glog_b[:, :w8 * HGRP]
                last_sig = nc.scalar.activation(
                    gate8[:, :w8 * HGRP], glog_in, AF.Sigmoid)
                hdc8 = spool.tile([TIL, GR * HGRP], F16, name="hdc8")
                if c1 != 0.0:
                    nc.vector.tensor_scalar_add(
                        hdc8[:, :w8 * HGRP], ghv[:, :, 1], c1)
                else:
                    nc.vector.tensor_copy(hdc8[:, :w8 * HGRP], ghv[:, :, 1])

                selg8 = spool.tile([TIL, GR * HGRP * S], F16, name="selg8")
                nc.vector.tensor_tensor(
                    out=selg8[:, :w8 * HGRP * S].rearrange(
                        "p (x j) -> p x j", j=S),
                    in0=sel8[:, :w8 * HGRP * S].rearrange(
                        "p (x j) -> p x j", j=S),
                    in1=gate8[:, :w8 * HGRP].rearrange(
                        "p (x one) -> p x one", one=1
                    ).to_broadcast([TIL, w8 * HGRP, S]),
                    op=AL.mult)

                for tt, t in enumerate(rd):
                    if t % VR == 0:
                        vdP = vdpool.tile([S, 8 * VR], F32, name="vdP")
                    for g in range(HGRP):
                        col = 8 * (t % VR) + g
                        x = HGRP * tt + g
                        nc.tensor.matmul(
                            vdP[0:S, col:col + 1],
                            lhsT=selg8[:, S * x:S * (x + 1)],
                            rhs=hdc8[:, x:x + 1],
                            start=True, stop=True)
                    if t % VR == VR - 1 or t == NLT - 1:
                        rv = t // VR
                        wid = HGRP * (t % VR + 1)
                        vstg = spool.tile([S, 8 * VR], F32, name="vstg")
                        nc.vector.tensor_copy(vstg[:, 0:wid], vdP[0:S, 0:wid])
                        wdmas.append(nc.sync.dma_start(
                            vdot3[:, TIL * rv:TIL * rv + wid, 0:1],
                            vstg[0:S, 0:wid].rearrange(
                                "j (t one) -> j t one", one=1)))

            # preload the Exp/Ln activation table while DMAs drain; keep it
            # AFTER the last sigmoid (the scheduler would otherwise hoist it
            # and force per-round sigmoid table reloads)
            dums = spool.tile([CHUNK, 2], F32, name="dums")
            dum_e = nc.scalar.activation(dums[:, 0:1], srcd[:, 0:1], AF.Exp)
            tile.add_dep_helper(dum_e.ins, last_sig.ins)
            dum_l = nc.scalar.activation(dums[:, 1:2], srcd[:, 0:1],
                                         AF.Ln, bias=1.0)
            tile.add_dep_helper(dum_l.ins, dum_e.ins)

            # ---- phase 2: gather 2 scalars per graph, logsigmoid ----
            vab = spool.tile([CHUNK, 16 * VW], F32, name="vab")
            for wi in range(16):
                g = nc.gpsimd.dma_gather(
                    out_ap=vab[:, VW * wi:VW * (wi + 1)].rearrange(
                        "p (one w) -> p one w", one=1),
                    in_ap=vdot_d[:, :],
                    idxs_ap=i16t[:, 8 * wi:8 * (wi + 1)],
                    num_idxs=CHUNK,
                    num_idxs_reg=CHUNK,
                    elem_size=VW)
                # wdmas complete in issue order (same queue), so one dep on
                # the last needed round covers all earlier rounds
                tile.add_dep_helper(g.ins, wdmas[qn[wi]].ins)
            vab3 = vab[:].rearrange("p (x w) -> p x w", w=VW)
            vs = spool.tile([CHUNK, NCH], F32, name="vs")
            nc.vector.tensor_add(vs[:], vab3[:, 0:NCH, 0], vab3[:, NCH:2 * NCH, 0])
            lg = spool.tile([CHUNK, NCH], F32, name="lg")
            nc.vector.tensor_add(lg[:], vs[:], srcd[:])
            if be0 != 0.0:
                lg2 = spool.tile([CHUNK, NCH], F32, name="lg2")
                nc.vector.tensor_scalar_add(lg2[:], lg[:], be0)
                lg = lg2
            x = spool.tile([CHUNK, NCH], F32, name="x")
            nc.vector.tensor_mul(x[:], lg[:], sgnb[:])
            mn = spool.tile([CHUNK, NCH], F32, name="mn")
            nc.vector.tensor_scalar_min(mn[:], x[:], 0.0)
            mx = spool.tile([CHUNK, NCH], F32, name="mx")
            nc.vector.tensor_scalar_max(mx[:], x[:], 0.0)
            nax = spool.tile([CHUNK, NCH], F32, name="nax")
            nc.vector.tensor_sub(nax[:], mn[:], mx[:])
            # logsigmoid(x) = min(x,0) - log1p(exp(-|x|))
            e = spool.tile([CHUNK, NCH], F32, name="e")
            nc.scalar.activation(e[:], nax[:], AF.Exp)
            lp = spool.tile([CHUNK, NCH], F32, name="lp")
            nc.scalar.activation(lp[:], e[:], AF.Ln, bias=1.0)
            outb = spool.tile([CHUNK, NCH], F32, name="outb")
            nc.vector.tensor_sub(outb[:], mn[:], lp[:])

            out_dst = out_d[:].rearrange("(p c) one -> p (c one)", p=CHUNK)
            nc.sync.dma_start(out_dst, outb[:])
    return nc


def _prep_core(hv16, seg_ids, last_idx, a, m):
    lo = int(np.searchsorted(seg_ids, m * BL, "left"))
    hi = int(np.searchsorted(seg_ids, (m + 1) * BL, "left"))
    nloc = hi - lo
    assert nloc <= NP, f"core {m}: {nloc} nodes > capacity {NP}"
    seg_loc = seg_ids[lo:hi].astype(np.int64) - m * BL

    hv_pad = np.zeros((NP, D), np.float16)
    hv_pad[:nloc] = hv16[lo:hi]
    # [NLT, 1024 nodes, 128 feat] -> transposed tiles [NLT, 128 feat, 1024 n]
    hvt = np.ascontiguousarray(
        hv_pad.reshape(NLT, TILB, D).transpose(0, 2, 1))

    nrt = (nloc + TIL - 1) // TIL
    bT = np.zeros(NTIL, np.int64)
    bT[:nrt] = seg_loc[np.arange(nrt) * TIL]
    segrel = np.full(NP, PAD_SEGREL, np.float32)
    rel = seg_loc - bT[np.arange(nloc) // TIL]
    assert rel.min() >= 0 and rel.max() < S, f"window overflow: {rel.max()}"
    segrel[:nloc] = rel
    sr_p = np.ascontiguousarray(segrel.reshape(NTIL, TIL).T.astype(np.float16))

    rr = np.arange(BL, dtype=np.int64)
    firsts = np.searchsorted(seg_loc, rr, "left")
    lasts = np.searchsorted(seg_loc, rr + 1, "left")
    nonempty = firsts < lasts
    th = firsts // TIL
    tl = np.maximum(lasts - 1, 0) // TIL
    assert np.all((tl - th)[nonempty] <= 1), "segment spans >2 tiles"
    j1 = rr - bT[th]
    assert np.all((j1[nonempty] >= 0) & (j1[nonempty] < S))
    # an unused (j, T) window slot is a guaranteed-zero vdot row
    used = set(zip(j1[nonempty].tolist(), th[nonempty].tolist()))
    straddle = nonempty & (tl > th)
    assert np.all(bT[tl[straddle]] == rr[straddle])
    used |= set(zip([0] * int(straddle.sum()), tl[straddle].tolist()))
    zrow = None
    for T in range(VR * HGRP):     # zero row must live in write-round 0
        for j in range(S):
            if (j, T) not in used:
                zrow = NTIL * j + T
                break
        if zrow is not None:
            break
    assert zrow is not None, "no unused round-0 window slot for zero row"
    idx1 = np.where(nonempty, NTIL * j1 + th, zrow).astype(np.int64)
    idx2 = np.where(straddle, tl, zrow).astype(np.int64)

    # last vdot-write round each gather depends on (zrow is round 0)
    qn = np.zeros(16, np.int64)
    grp = VR * HGRP
    for c in range(NCH):
        sl = slice(CHUNK * c, CHUNK * (c + 1))
        ne, st = nonempty[sl], straddle[sl]
        qn[c] = (th[sl][ne].max() // grp) if ne.any() else 0
        qn[NCH + c] = (tl[sl][st].max() // grp) if st.any() else 0

    # dma_gather idx layout: idx k of gather w at [k % 16, 8w + k // 16],
    # replicated across the 8 GPSIMD cores' 16-partition stripes
    i16t = np.zeros((CHUNK, 2 * NCH * 8), np.int16)
    for c in range(NCH):
        blk = idx1[CHUNK * c:CHUNK * (c + 1)].reshape(8, 16).T
        i16t[:, 8 * c:8 * (c + 1)] = np.tile(blk, (8, 1))
        blk2 = idx2[CHUNK * c:CHUNK * (c + 1)].reshape(8, 16).T
        i16t[:, 8 * (NCH + c):8 * (NCH + c + 1)] = np.tile(blk2, (8, 1))

    src = hv16[last_idx[m * BL:(m + 1) * BL]]
    src_p = np.ascontiguousarray(
        src.reshape(NCH, CHUNK, D).transpose(1, 0, 2).reshape(CHUNK, NCH * D))
    sgn = (2 * a[m * BL:(m + 1) * BL] - 1).astype(np.float32)
    sgn_p = np.ascontiguousarray(sgn.reshape(NCH, CHUNK).T)
    return hvt, sr_p, i16t, src_p, sgn_p, qn


def prep_all(hv, Wg, bg, Wp, bp, We, be, seg_ids, last_idx, a):
    """Host-side sharding/folding. Returns (in_maps, bg0, be0, c1)."""
    hv = np.asarray(hv, dtype=np.float32)
    Wg = np.asarray(Wg, dtype=np.float32)
    bg = np.asarray(bg, dtype=np.float32)
    Wp = np.asarray(Wp, dtype=np.float32)
    bp = np.asarray(bp, dtype=np.float32)
    We = np.asarray(We, dtype=np.float32)
    be = np.asarray(be, dtype=np.float32)
    seg_ids = np.asarray(seg_ids)
    last_idx = np.asarray(last_idx)
    a = np.asarray(a)

    G = 2 * D
    w1 = (Wp @ We[:G]).astype(np.float32)[:, 0]        # [128]
    wes = We[G:, 0].astype(np.float32)                 # [128]
    c1 = float(bp @ We[:G, 0])
    bg0, be0 = float(bg[0]), float(be[0])

    cst = np.zeros((TIL, C_W), np.float16)
    cst[:, C_WGW1] = Wg[:, 0].astype(np.float16)
    cst[:, C_WGW1 + 1] = w1.astype(np.float16)
    slot = np.arange(S, dtype=np.float32)
    cst[:, C_IOTA:C_IOTA + GR * HGRP * S] = np.tile(slot, GR * HGRP)[None, :]
    cst[:, C_WES:C_WES + D] = wes[None, :]

    hv16 = hv.astype(np.float16)
    in_maps = []
    qn = np.zeros(16, np.int64)
    for m in range(NCORES):
        hvt, sr_p, i16t, src_p, sgn_p, qn_m = _prep_core(
            hv16, seg_ids, last_idx, a, m)
        qn = np.maximum(qn, qn_m)
        in_maps.append({
            "hvt": hvt, "segrel": sr_p, "cst": cst, "src": src_p,
            "i16t": i16t, "sgn": sgn_p,
        })
    return in_maps, bg0, be0, c1, [int(v) for v in qn]


def kernel(hv, Wg, bg, Wp, bp, We, be, seg_ids, last_idx, a):
    global LAST_RESULTS, LAST_NC
    in_maps, bg0, be0, c1, qn = prep_all(
        hv, Wg, bg, Wp, bp, We, be, seg_ids, last_idx, a)
    nc = _build(bg0, be0, c1, qn)
    split_sync_waits(nc, maxw=1)
    lower_extended_insts(nc)
    LAST_NC = nc
    res = run_bass_kernel_spmd(nc, in_maps, core_ids=list(range(NCORES)))
    LAST_RESULTS = res
    out = np.concatenate(
        [np.asarray(res.results[i]["out"]).reshape(CHUNK, NCH).T.reshape(-1, 1)
         for i in range(NCORES)], axis=0)
    return out.astype(np.float32)
